# revision 22
# baseline (speedup 1.0000x reference)
"""Trainium2 Bass kernel for nn_MetricNet (512-step elementwise Euler recurrence).

Strategy: pure data parallel over the batch axis — each of the 8 NeuronCores
gets 16384 frequencies as a [128 x 128] f32 SBUF tile held for all 512 steps.

The recurrence runs as THREE custom DVE uop programs per step (the stock
ALU set needs four DVE ops plus two ACT ops):

  T1_j     = (Yc_j + d_j) * (T1_{j-1} + ktd_{j-1})    [ANT_TT_AA]
  GG_j     = (sq(T1_j + kt_j) - S0_j) * W             [ANT_QGW]
  Yc_{j+1} = sq(Yc_j + d_j) * 0.5 - GG_j              [ANT_YSQ]

ANT_TT_AA folds the U-state update away entirely (U_{j+1} = T1_j + ktd_j
is substituted into the next step's product, so U never materialises);
ANT_QGW fuses the square, the S0 shift and the W multiply (S0 = 1/b1^2
exactly — the inv1^2 and inv2/p terms cancel); ANT_YSQ fuses the
(Y+c1)^2/2 term so the Scalar engine drops out of the loop completely.
The Yc gauge (Y_j = Yc_j + e_j with e_{j+1} = sigma_j - c1_j^2/2,
d_j = e_j + c1_j) absorbs every additive per-step constant. Each op is
split into two independent half-width [128,64] ops, interleaved A/B so
consecutive DVE instructions never have a direct data hazard. The loop
is single-engine: no semaphores, 3072 back-to-back DVE instructions.

Final outputs are assembled on the host in f64 during unsharding:
Re = T1_last + ktd_last (the last ktd carries delta = -inv1 so this IS
Re), Im = (Yc_final + e_final)/m with m = 2*dz*omega.
"""

import numpy as np

import concourse.bass as bass
import concourse.mybir as mybir
import bass_rust as _br
from concourse import tile
from concourse.bass_utils import run_bass_kernel_spmd

# walrus's codegen rejects instructions carrying more than ~2 sync-wait
# commands, but Tile's exit path hangs the full end-of-kernel wait set
# (one per engine/DMA lane used) on a single SP drain. Split those waits
# across dedicated one-wait NOPs ahead of a bare drain instead.
_orig_drain_and_barrier = tile.TileContext._drain_and_barrier


def _split_drain_and_barrier(self, tick_clock, wait_clock):
    nc = self.nc
    probe = nc.sync.nop()
    wait_clock.add_sem_waits(
        probe.ins, _br.ScopedClock({None: tick_clock.global_clock})
    )
    si = probe.ins.sync_info
    if si is not None and len(si.on_wait) > 1:
        waits = list(si.on_wait)
        probe.ins.sync_info = _br.SyncInfo(
            on_wait=waits[:1], on_update=list(si.on_update)
        )
        for w in waits[1:]:
            extra = nc.sync.nop()
            extra.ins.sync_info = _br.SyncInfo(on_wait=[w], on_update=[])
    nc.sync.drain()
    nc.all_engine_barrier()
    popped = nc._tile_sem_poison_stack.pop()
    assert popped is self._sem_poison
    nc.clear_and_free_semaphores(list(self.sems.allocated().values()))
    nc.all_engine_barrier()


tile.TileContext._drain_and_barrier = _split_drain_and_barrier

# This kernel only uses DVE (+SP for DMA); the stock all-engine barrier
# waits on every engine's cold wake (~2.6us for PE alone). Restrict full
# barriers to the two engines that actually execute instructions.
_orig_all_engine_barrier = bass.Bass.all_engine_barrier


def _no_pe_all_engine_barrier(self, *, sem_only: bool = False):
    if sem_only:
        return _orig_all_engine_barrier(self, sem_only=True)
    engs = [e for e in self.engines if e != mybir.EngineType.PE]
    self.multi_engine_barrier(engs)


bass.Bass.all_engine_barrier = _no_pe_all_engine_barrier


def _hoist_extra_waits(nc):
    """walrus's per-instruction sync-wait budget is 1 for compute/DMA
    instructions (2 for TPB_CTRL). Hoist surplus waits onto same-engine NOPs
    spliced immediately before the over-budget instruction — the engine
    executes in order, so waiting earlier is semantically identical."""
    for bb in nc.main_func.blocks:
        insts = bb.instructions
        out = []
        changed = False
        for ins in insts:
            si = ins.sync_info
            if si is not None and len(si.on_wait) > 1:
                waits = list(si.on_wait)
                for w in waits[:-1]:
                    nop = mybir.InstNoOp(
                        name=nc.get_next_instruction_name(),
                        engine=ins.engine,
                        sync_info=_br.SyncInfo(on_wait=[w], on_update=[]),
                    )
                    nc.register_instruction(nop)
                    out.append(nop)
                ins.sync_info = _br.SyncInfo(
                    on_wait=waits[-1:], on_update=list(si.on_update)
                )
                changed = True
            out.append(ins)
        if changed:
            bb.instructions = out


def _hoist_input_dmas(nc):
    """The input DMACopies carry no sync waits — they are gated only by
    sitting after the entry barrier in program order, which costs ~5us of
    engine-wake/preamble before the ~2.2us SWDGE issue latency even starts.
    Move them into the preamble block, right after their engine's boot
    RegisterMoves, so the transfer overlaps the barrier. (The preamble only
    memsets const-AP tiles, never the DMA destination.)"""
    blocks = nc.main_func.blocks
    if len(blocks) < 2:
        return
    b0, b1 = blocks[0], blocks[1]
    moved = []
    keep = []
    for ins in b1.instructions:
        si = ins.sync_info
        if (ins.opcode == "DMACopy" and (si is None or not si.on_wait)
                and len(moved) < 3):
            moved.append(ins)
        else:
            keep.append(ins)
    if not moved:
        return
    b1.instructions = keep
    new0 = []
    inserted = set()
    # insert each moved DMA after the LAST RegisterMove of its engine
    last_rm = {}
    for i, ins in enumerate(b0.instructions):
        if ins.opcode == "RegisterMove":
            last_rm[ins.engine] = i
    order = sorted(moved, key=lambda m: -last_rm.get(m.engine, 0))
    insts = list(b0.instructions)
    for m in order:
        pos = last_rm.get(m.engine)
        if pos is None:
            insts.append(m)
        else:
            insts.insert(pos + 1, m)
    b0.instructions = insts


def _strip_loop_ticks(nc):
    """Every loop DVE instruction self-waits and +1-increments the Tile tick
    semaphore — pure bookkeeping on an in-order, single-engine loop where
    only the final output-DMA/drain waits consume the tick. Strip the waits
    and updates from all but the last 8 loop instructions and fold the
    stripped increments into a bulk update on the first kept one, so every
    surviving threshold fires at exactly the original tick values."""
    import bass_rust as br
    blocks = nc.main_func.blocks
    b1 = blocks[1]
    isa = [i for i in b1.instructions
           if i.engine == mybir.EngineType.DVE and i.opcode == "ISA"
           and i.sync_info is not None]
    if len(isa) < 16:
        return
    # identify the tick sem: the sem this stream increments
    tick = None
    for i in isa:
        for u in i.sync_info.on_update:
            if u.update_mode == "sem-inc":
                tick = u.ant_name
                break
        if tick:
            break
    if tick is None:
        return
    # Variant A: strip only the redundant self-WAITS on the tick sem
    # (same-engine in-order execution makes them tautological); keep every
    # +1 update so all downstream thresholds are untouched.
    for i in isa:
        si = i.sync_info
        w = [x for x in si.on_wait if x.ant_name != tick]
        if len(w) != len(si.on_wait):
            i.sync_info = br.SyncInfo(
                on_wait=w, on_update=list(si.on_update)
            )


def _register_custom_ops():
    """Register the three fused DVE uop programs (idempotent)."""
    import concourse.dve_ops as dve_ops
    from concourse.dve_spec import Spec, Src0, Src1, C0, C1, lower, sq
    from concourse.dve_spec import _has_src1 as has_src1
    from concourse.dve_uop import DveOpSpec

    existing = {op.name: op for op in dve_ops.OPS}
    if "ANT_TT_AA" in existing:
        return (existing["ANT_TT_AA"], existing["ANT_QGW"],
                existing["ANT_YSQ"])

    def make(name, body, ref):
        spec = Spec(body=body, reference=ref)
        row = dve_ops._CUSTOM_DVE_ROW_BASE + len(dve_ops.OPS)
        shas = {}
        for ver in ("v3", "v4"):
            try:
                uops = lower(spec, ver=ver)
                shas[ver] = DveOpSpec(
                    name=name, opcode=row, uops=uops, rd1_en=has_src1(spec)
                ).sha(ver)
            except Exception:
                pass
        op = dve_ops.DveOp(name, spec, subdim=False, uops_sha=shas)
        dve_ops.OPS.append(op)
        dve_ops._SUB_OPCODE_FOR_NAME[name] = row
        dve_ops.CUSTOM_DVE_SPECS[name] = spec
        return op

    tt_aa = make(
        "ANT_TT_AA",
        (Src0 + C0) * (Src1 + C1),
        lambda in0, in1, s0, s1, imm2: (in0.astype(np.float32) + s0)
        * (in1 + s1),
    )
    qgw = make(
        "ANT_QGW",
        (sq(Src0 + C0) + C1) * Src1,
        lambda in0, in1, s0, s1, imm2: (
            ((in0.astype(np.float32) + s0) ** 2 + s1) * in1
        ),
    )
    ysq = make(
        "ANT_YSQ",
        sq(Src0 + C0) * C1 - Src1,
        lambda in0, in1, s0, s1, imm2: (
            (in0.astype(np.float32) + s0) ** 2 * s1 - in1
        ),
    )
    return tt_aa, qgw, ysq


N_LAYERS = 512
Z_INI = 0.0
DEL_Z = 0.9 / 512.0
MU = 1.0
BATCH = 131072
N_CORES = 8
P = 128
F = BATCH // N_CORES // P  # 128

F32 = mybir.dt.float32
ALU = mybir.AluOpType


def _host_scalars(B: np.ndarray, p: float):
    """Per-step scalar schedule, float64."""
    n = N_LAYERS
    zs = Z_INI + DEL_Z * np.arange(n, dtype=np.float64)
    b1 = B.astype(np.float64)[:n]
    b2 = B.astype(np.float64)[1 : n + 1]
    c1 = 2.0 - b2 / b1  # 1 + g
    inv1 = 1.0 / (p * (1.0 - zs))
    inv2 = inv1 / (1.0 - zs)
    kt = -DEL_Z * inv2
    delta = np.empty(n)
    delta[:-1] = inv1[1:] - inv1[:-1]
    delta[-1] = -inv1[-1]  # so the final U update yields Re_out exactly
    ktd = kt + delta
    # S0 = -inv2/p + inv1^2 + 1/b1^2; the first two cancel exactly
    S0 = 1.0 / (b1 * b1)
    sigma = -2.0 * DEL_Z * DEL_Z * zs * zs * (MU * MU) / b1
    e = np.zeros(n + 1)
    e[1:] = sigma - 0.5 * c1 * c1  # gauge offset: Y_j = Yc_j + e_j
    d = e[:n] + c1
    return c1, kt, ktd, S0, e, d, inv1


def _build_bass(tt_aa, qgw, ysq, d, kt, ktd, S0):
    n = N_LAYERS
    nc = bass.Bass()
    # packed input: [U0 | Yc0 | W] (host-prepared)
    x_in = nc.dram_tensor("x_in", [P, 3 * F], F32, kind="ExternalInput")
    # packed output: [T1_last | Yc_final] (host assembles Re/Im)
    x_out = nc.dram_tensor("x_out", [P, 2 * F], F32, kind="ExternalOutput")

    f = float  # immediates
    with tile.TileContext(nc) as tc:
        with tc.tile_pool(name="pool", bufs=1) as pool:
            xin = pool.tile([P, 3 * F], F32)
            # Split the input DMA: U0/Y0 (needed by step 0's first ops) on
            # the SP queue; W (first read ~270ns into the loop) in parallel
            # on the otherwise-idle ACT queue.
            nc.sync.dma_start(xin[:, 0 : 2 * F], x_in[:, 0 : 2 * F])
            nc.scalar.dma_start(xin[:, 2 * F : 3 * F], x_in[:, 2 * F : 3 * F])
            U0 = xin[:, 0:F]
            Y0 = xin[:, F : 2 * F]
            W = xin[:, 2 * F : 3 * F]

            Ya = pool.tile([P, F], F32)
            Yb = pool.tile([P, F], F32)
            Ta = pool.tile([P, F], F32)
            Tb = pool.tile([P, F], F32)
            GG = pool.tile([P, F], F32)
            xout = pool.tile([P, 2 * F], F32)
            t1o = xout[:, 0:F]
            yco = xout[:, F : 2 * F]

            v = nc.vector
            hA = slice(0, F // 2)
            hB = slice(F // 2, F)
            Ys, Yn = Ya, Yb
            T1, T1p = Ta, Tb
            for j in range(n):
                ys_src = Y0 if j == 0 else Ys[:]
                t1p_src = U0 if j == 0 else T1p[:]
                ktd_prev = 0.0 if j == 0 else f(ktd[j - 1])
                t1_dst = t1o if j == n - 1 else T1[:]
                yn_dst = yco if j == n - 1 else Yn[:]
                dj = f(d[j])
                # T1 = (Yc + d_j)*(T1p + ktd_{j-1})
                v._custom_dve(tt_aa, out=t1_dst[:, hA], in0=ys_src[:, hA],
                              in1=t1p_src[:, hA], s0=dj, s1=ktd_prev)
                v._custom_dve(tt_aa, out=t1_dst[:, hB], in0=ys_src[:, hB],
                              in1=t1p_src[:, hB], s0=dj, s1=ktd_prev)
                # GG = (sq(T1 + kt_j) - S0_j)*W
                v._custom_dve(qgw, out=GG[:, hA], in0=t1_dst[:, hA],
                              in1=W[:, hA], s0=f(kt[j]), s1=f(-S0[j]))
                v._custom_dve(qgw, out=GG[:, hB], in0=t1_dst[:, hB],
                              in1=W[:, hB], s0=f(kt[j]), s1=f(-S0[j]))
                # Yc' = sq(Yc + d_j)*0.5 - GG
                v._custom_dve(ysq, out=yn_dst[:, hA], in0=ys_src[:, hA],
                              in1=GG[:, hA], s0=dj, s1=0.5)
                v._custom_dve(ysq, out=yn_dst[:, hB], in0=ys_src[:, hB],
                              in1=GG[:, hB], s0=dj, s1=0.5)
                Ys, Yn = Yn, Ys
                T1, T1p = T1p, T1

            # Split the output DMA: T1 finalises 4 slices (~540ns) before
            # Yc, so its descriptor-gen overlaps the loop tail on the idle
            # ACT queue; Yc goes on SP.
            nc.scalar.dma_start(x_out[:, 0:F], xout[:, 0:F])
            nc.sync.dma_start(x_out[:, F : 2 * F], xout[:, F : 2 * F])
    _hoist_extra_waits(nc)
    _strip_loop_ticks(nc)
    _hoist_input_dmas(nc)
    mybir.codegen_inst_isa_subclasses(nc)
    return nc


def kernel(Re_s, Im_s, omega, PiT, B, _trace=False):
    Re_s = np.ascontiguousarray(Re_s, dtype=np.float32)
    Im_s = np.ascontiguousarray(Im_s, dtype=np.float32)
    omega = np.ascontiguousarray(omega, dtype=np.float32)
    p = float(np.asarray(PiT).reshape(-1)[0])
    n = N_LAYERS
    tt_aa, qgw, ysq = _register_custom_ops()
    c1, kt, ktd, S0, e, d, inv1 = _host_scalars(np.asarray(B), p)

    nc = _build_bass(tt_aa, qgw, ysq, d, kt, ktd, S0)

    m64 = 2.0 * DEL_Z * omega.astype(np.float64)
    U0 = (Re_s.astype(np.float64) + inv1[0]).astype(np.float32)
    Y0 = (Im_s.astype(np.float64) * m64).astype(np.float32)
    Wf = (0.5 * m64 * m64).astype(np.float32)
    U08 = U0.reshape(N_CORES, P, F)
    Y08 = Y0.reshape(N_CORES, P, F)
    W8 = Wf.reshape(N_CORES, P, F)
    xin = np.concatenate([U08, Y08, W8], axis=2)  # [8, P, 3F]
    in_maps = [{"x_in": np.ascontiguousarray(xin[i])} for i in range(N_CORES)]
    res = run_bass_kernel_spmd(nc, in_maps, list(range(N_CORES)), trace=_trace)
    t1_full = np.concatenate(
        [res.results[i]["x_out"][:, 0:F].reshape(-1) for i in range(N_CORES)]
    )
    yc_full = np.concatenate(
        [res.results[i]["x_out"][:, F : 2 * F].reshape(-1) for i in range(N_CORES)]
    )
    re_full = t1_full.astype(np.float64) + ktd[n - 1]
    im_full = (yc_full.astype(np.float64) + e[n]) / m64
    if _trace:
        kernel.last_results = res
    return re_full.astype(np.float32), im_full.astype(np.float32)


# revision 23
# speedup vs baseline: 1.0021x; 1.0021x over previous
"""Trainium2 Bass kernel for nn_MetricNet (512-step elementwise Euler recurrence).

Strategy: pure data parallel over the batch axis — each of the 8 NeuronCores
gets 16384 frequencies as a [128 x 128] f32 SBUF tile held for all 512 steps.

The recurrence runs as THREE custom DVE uop programs per step (the stock
ALU set needs four DVE ops plus two ACT ops):

  T1_j     = (Yc_j + d_j) * (T1_{j-1} + ktd_{j-1})    [ANT_TT_AA]
  GG_j     = (sq(T1_j + kt_j) - S0_j) * W             [ANT_QGW]
  Yc_{j+1} = sq(Yc_j + d_j) * 0.5 - GG_j              [ANT_YSQ]

ANT_TT_AA folds the U-state update away entirely (U_{j+1} = T1_j + ktd_j
is substituted into the next step's product, so U never materialises);
ANT_QGW fuses the square, the S0 shift and the W multiply (S0 = 1/b1^2
exactly — the inv1^2 and inv2/p terms cancel); ANT_YSQ fuses the
(Y+c1)^2/2 term so the Scalar engine drops out of the loop completely.
The Yc gauge (Y_j = Yc_j + e_j with e_{j+1} = sigma_j - c1_j^2/2,
d_j = e_j + c1_j) absorbs every additive per-step constant. Each op is
split into two independent half-width [128,64] ops, interleaved A/B so
consecutive DVE instructions never have a direct data hazard. The loop
is single-engine: no semaphores, 3072 back-to-back DVE instructions.

Final outputs are assembled on the host in f64 during unsharding:
Re = T1_last + ktd_last (the last ktd carries delta = -inv1 so this IS
Re), Im = (Yc_final + e_final)/m with m = 2*dz*omega.
"""

import numpy as np

import concourse.bass as bass
import concourse.mybir as mybir
import bass_rust as _br
from concourse import tile
from concourse.bass_utils import run_bass_kernel_spmd

# walrus's codegen rejects instructions carrying more than ~2 sync-wait
# commands, but Tile's exit path hangs the full end-of-kernel wait set
# (one per engine/DMA lane used) on a single SP drain. Split those waits
# across dedicated one-wait NOPs ahead of a bare drain instead.
_orig_drain_and_barrier = tile.TileContext._drain_and_barrier


def _split_drain_and_barrier(self, tick_clock, wait_clock):
    nc = self.nc
    probe = nc.sync.nop()
    wait_clock.add_sem_waits(
        probe.ins, _br.ScopedClock({None: tick_clock.global_clock})
    )
    si = probe.ins.sync_info
    if si is not None and len(si.on_wait) > 1:
        waits = list(si.on_wait)
        probe.ins.sync_info = _br.SyncInfo(
            on_wait=waits[:1], on_update=list(si.on_update)
        )
        for w in waits[1:]:
            extra = nc.sync.nop()
            extra.ins.sync_info = _br.SyncInfo(on_wait=[w], on_update=[])
    nc.sync.drain()
    nc.all_engine_barrier()
    popped = nc._tile_sem_poison_stack.pop()
    assert popped is self._sem_poison
    nc.clear_and_free_semaphores(list(self.sems.allocated().values()))
    nc.all_engine_barrier()


tile.TileContext._drain_and_barrier = _split_drain_and_barrier

# This kernel only uses DVE (+SP for DMA); the stock all-engine barrier
# waits on every engine's cold wake (~2.6us for PE alone). Restrict full
# barriers to the two engines that actually execute instructions.
_orig_all_engine_barrier = bass.Bass.all_engine_barrier


def _no_pe_all_engine_barrier(self, *, sem_only: bool = False):
    if sem_only:
        return _orig_all_engine_barrier(self, sem_only=True)
    engs = [e for e in self.engines if e != mybir.EngineType.PE]
    self.multi_engine_barrier(engs)


bass.Bass.all_engine_barrier = _no_pe_all_engine_barrier


def _hoist_extra_waits(nc):
    """walrus's per-instruction sync-wait budget is 1 for compute/DMA
    instructions (2 for TPB_CTRL). Hoist surplus waits onto same-engine NOPs
    spliced immediately before the over-budget instruction — the engine
    executes in order, so waiting earlier is semantically identical."""
    for bb in nc.main_func.blocks:
        insts = bb.instructions
        out = []
        changed = False
        for ins in insts:
            si = ins.sync_info
            if si is not None and len(si.on_wait) > 1:
                waits = list(si.on_wait)
                for w in waits[:-1]:
                    nop = mybir.InstNoOp(
                        name=nc.get_next_instruction_name(),
                        engine=ins.engine,
                        sync_info=_br.SyncInfo(on_wait=[w], on_update=[]),
                    )
                    nc.register_instruction(nop)
                    out.append(nop)
                ins.sync_info = _br.SyncInfo(
                    on_wait=waits[-1:], on_update=list(si.on_update)
                )
                changed = True
            out.append(ins)
        if changed:
            bb.instructions = out


def _hoist_input_dmas(nc):
    """The input DMACopies carry no sync waits — they are gated only by
    sitting after the entry barrier in program order, which costs ~5us of
    engine-wake/preamble before the ~2.2us SWDGE issue latency even starts.
    Move them into the preamble block, right after their engine's boot
    RegisterMoves, so the transfer overlaps the barrier. (The preamble only
    memsets const-AP tiles, never the DMA destination.)"""
    blocks = nc.main_func.blocks
    if len(blocks) < 2:
        return
    b0, b1 = blocks[0], blocks[1]
    moved = []
    keep = []
    for ins in b1.instructions:
        si = ins.sync_info
        if (ins.opcode == "DMACopy" and (si is None or not si.on_wait)
                and len(moved) < 3):
            moved.append(ins)
        else:
            keep.append(ins)
    if not moved:
        return
    b1.instructions = keep
    new0 = []
    inserted = set()
    # insert each moved DMA after the LAST RegisterMove of its engine
    last_rm = {}
    for i, ins in enumerate(b0.instructions):
        if ins.opcode == "RegisterMove":
            last_rm[ins.engine] = i
    order = sorted(moved, key=lambda m: -last_rm.get(m.engine, 0))
    insts = list(b0.instructions)
    for m in order:
        pos = last_rm.get(m.engine)
        if pos is None:
            insts.append(m)
        else:
            insts.insert(pos + 1, m)
    b0.instructions = insts


def _strip_loop_ticks(nc):
    """Every loop DVE instruction self-waits and +1-increments the Tile tick
    semaphore — pure bookkeeping on an in-order, single-engine loop where
    only the final output-DMA/drain waits consume the tick. Strip the waits
    and updates from all but the last 8 loop instructions and fold the
    stripped increments into a bulk update on the first kept one, so every
    surviving threshold fires at exactly the original tick values."""
    import bass_rust as br
    blocks = nc.main_func.blocks
    b1 = blocks[1]
    isa = [i for i in b1.instructions
           if i.engine == mybir.EngineType.DVE and i.opcode == "ISA"
           and i.sync_info is not None]
    if len(isa) < 16:
        return
    # identify the tick sem: the sem this stream increments
    tick = None
    for i in isa:
        for u in i.sync_info.on_update:
            if u.update_mode == "sem-inc":
                tick = u.ant_name
                break
        if tick:
            break
    if tick is None:
        return
    # Strip tick self-waits AND +1 updates from the loop body (in-order
    # single-engine execution makes both redundant); keep the last 8 ops'
    # updates and renumber every downstream tick threshold by the stripped
    # count — the monotone mapping preserves exact firing order (the early
    # t1o-DMA still fires at the final T1 write).
    keep_tail = 8
    body, tail = isa[:-keep_tail], isa[-keep_tail:]
    stripped = 0
    for i in body:
        si = i.sync_info
        w = [x for x in si.on_wait if x.ant_name != tick]
        u = [x for x in si.on_update if x.ant_name != tick]
        stripped += len(si.on_update) - len(u)
        if not w and not u:
            i.sync_info = None
        else:
            i.sync_info = br.SyncInfo(on_wait=w, on_update=u)
    for i in tail:
        si = i.sync_info
        w = [x for x in si.on_wait if x.ant_name != tick]
        if len(w) != len(si.on_wait):
            i.sync_info = br.SyncInfo(on_wait=w, on_update=list(si.on_update))
    # renumber every remaining wait on the tick sem, program-wide
    for bb in blocks:
        for i in bb.instructions:
            si = i.sync_info
            if si is None or not si.on_wait:
                continue
            hit = [x for x in si.on_wait if x.ant_name == tick]
            if not hit:
                continue
            keepw = []
            for x in si.on_wait:
                if x.ant_name == tick:
                    nv = x.wait_value - stripped
                    if nv <= 0:
                        continue  # trivially satisfied
                    x.wait_value = nv
                keepw.append(x)
            if not keepw and not si.on_update:
                i.sync_info = None
            else:
                i.sync_info = br.SyncInfo(
                    on_wait=keepw, on_update=list(si.on_update)
                )


def _register_custom_ops():
    """Register the three fused DVE uop programs (idempotent)."""
    import concourse.dve_ops as dve_ops
    from concourse.dve_spec import Spec, Src0, Src1, C0, C1, lower, sq
    from concourse.dve_spec import _has_src1 as has_src1
    from concourse.dve_uop import DveOpSpec

    existing = {op.name: op for op in dve_ops.OPS}
    if "ANT_TT_AA" in existing:
        return (existing["ANT_TT_AA"], existing["ANT_QGW"],
                existing["ANT_YSQ"])

    def make(name, body, ref):
        spec = Spec(body=body, reference=ref)
        row = dve_ops._CUSTOM_DVE_ROW_BASE + len(dve_ops.OPS)
        shas = {}
        for ver in ("v3", "v4"):
            try:
                uops = lower(spec, ver=ver)
                shas[ver] = DveOpSpec(
                    name=name, opcode=row, uops=uops, rd1_en=has_src1(spec)
                ).sha(ver)
            except Exception:
                pass
        op = dve_ops.DveOp(name, spec, subdim=False, uops_sha=shas)
        dve_ops.OPS.append(op)
        dve_ops._SUB_OPCODE_FOR_NAME[name] = row
        dve_ops.CUSTOM_DVE_SPECS[name] = spec
        return op

    tt_aa = make(
        "ANT_TT_AA",
        (Src0 + C0) * (Src1 + C1),
        lambda in0, in1, s0, s1, imm2: (in0.astype(np.float32) + s0)
        * (in1 + s1),
    )
    qgw = make(
        "ANT_QGW",
        (sq(Src0 + C0) + C1) * Src1,
        lambda in0, in1, s0, s1, imm2: (
            ((in0.astype(np.float32) + s0) ** 2 + s1) * in1
        ),
    )
    ysq = make(
        "ANT_YSQ",
        sq(Src0 + C0) * C1 - Src1,
        lambda in0, in1, s0, s1, imm2: (
            (in0.astype(np.float32) + s0) ** 2 * s1 - in1
        ),
    )
    return tt_aa, qgw, ysq


N_LAYERS = 512
Z_INI = 0.0
DEL_Z = 0.9 / 512.0
MU = 1.0
BATCH = 131072
N_CORES = 8
P = 128
F = BATCH // N_CORES // P  # 128

F32 = mybir.dt.float32
ALU = mybir.AluOpType


def _host_scalars(B: np.ndarray, p: float):
    """Per-step scalar schedule, float64."""
    n = N_LAYERS
    zs = Z_INI + DEL_Z * np.arange(n, dtype=np.float64)
    b1 = B.astype(np.float64)[:n]
    b2 = B.astype(np.float64)[1 : n + 1]
    c1 = 2.0 - b2 / b1  # 1 + g
    inv1 = 1.0 / (p * (1.0 - zs))
    inv2 = inv1 / (1.0 - zs)
    kt = -DEL_Z * inv2
    delta = np.empty(n)
    delta[:-1] = inv1[1:] - inv1[:-1]
    delta[-1] = -inv1[-1]  # so the final U update yields Re_out exactly
    ktd = kt + delta
    # S0 = -inv2/p + inv1^2 + 1/b1^2; the first two cancel exactly
    S0 = 1.0 / (b1 * b1)
    sigma = -2.0 * DEL_Z * DEL_Z * zs * zs * (MU * MU) / b1
    e = np.zeros(n + 1)
    e[1:] = sigma - 0.5 * c1 * c1  # gauge offset: Y_j = Yc_j + e_j
    d = e[:n] + c1
    return c1, kt, ktd, S0, e, d, inv1


def _build_bass(tt_aa, qgw, ysq, d, kt, ktd, S0):
    n = N_LAYERS
    nc = bass.Bass()
    # packed input: [U0 | Yc0 | W] (host-prepared)
    x_in = nc.dram_tensor("x_in", [P, 3 * F], F32, kind="ExternalInput")
    # packed output: [T1_last | Yc_final] (host assembles Re/Im)
    x_out = nc.dram_tensor("x_out", [P, 2 * F], F32, kind="ExternalOutput")

    f = float  # immediates
    with tile.TileContext(nc) as tc:
        with tc.tile_pool(name="pool", bufs=1) as pool:
            xin = pool.tile([P, 3 * F], F32)
            # Split the input DMA: U0/Y0 (needed by step 0's first ops) on
            # the SP queue; W (first read ~270ns into the loop) in parallel
            # on the otherwise-idle ACT queue.
            nc.sync.dma_start(xin[:, 0 : 2 * F], x_in[:, 0 : 2 * F])
            nc.scalar.dma_start(xin[:, 2 * F : 3 * F], x_in[:, 2 * F : 3 * F])
            U0 = xin[:, 0:F]
            Y0 = xin[:, F : 2 * F]
            W = xin[:, 2 * F : 3 * F]

            Ya = pool.tile([P, F], F32)
            Yb = pool.tile([P, F], F32)
            Ta = pool.tile([P, F], F32)
            Tb = pool.tile([P, F], F32)
            GG = pool.tile([P, F], F32)
            xout = pool.tile([P, 2 * F], F32)
            t1o = xout[:, 0:F]
            yco = xout[:, F : 2 * F]

            v = nc.vector
            hA = slice(0, F // 2)
            hB = slice(F // 2, F)
            Ys, Yn = Ya, Yb
            T1, T1p = Ta, Tb
            for j in range(n):
                ys_src = Y0 if j == 0 else Ys[:]
                t1p_src = U0 if j == 0 else T1p[:]
                ktd_prev = 0.0 if j == 0 else f(ktd[j - 1])
                t1_dst = t1o if j == n - 1 else T1[:]
                yn_dst = yco if j == n - 1 else Yn[:]
                dj = f(d[j])
                # T1 = (Yc + d_j)*(T1p + ktd_{j-1})
                v._custom_dve(tt_aa, out=t1_dst[:, hA], in0=ys_src[:, hA],
                              in1=t1p_src[:, hA], s0=dj, s1=ktd_prev)
                v._custom_dve(tt_aa, out=t1_dst[:, hB], in0=ys_src[:, hB],
                              in1=t1p_src[:, hB], s0=dj, s1=ktd_prev)
                # GG = (sq(T1 + kt_j) - S0_j)*W
                v._custom_dve(qgw, out=GG[:, hA], in0=t1_dst[:, hA],
                              in1=W[:, hA], s0=f(kt[j]), s1=f(-S0[j]))
                v._custom_dve(qgw, out=GG[:, hB], in0=t1_dst[:, hB],
                              in1=W[:, hB], s0=f(kt[j]), s1=f(-S0[j]))
                # Yc' = sq(Yc + d_j)*0.5 - GG
                v._custom_dve(ysq, out=yn_dst[:, hA], in0=ys_src[:, hA],
                              in1=GG[:, hA], s0=dj, s1=0.5)
                v._custom_dve(ysq, out=yn_dst[:, hB], in0=ys_src[:, hB],
                              in1=GG[:, hB], s0=dj, s1=0.5)
                Ys, Yn = Yn, Ys
                T1, T1p = T1p, T1

            # Split the output DMA: T1 finalises 4 slices (~540ns) before
            # Yc, so its descriptor-gen overlaps the loop tail on the idle
            # ACT queue; Yc goes on SP.
            nc.scalar.dma_start(x_out[:, 0:F], xout[:, 0:F])
            nc.sync.dma_start(x_out[:, F : 2 * F], xout[:, F : 2 * F])
    _hoist_extra_waits(nc)
    _strip_loop_ticks(nc)
    _hoist_input_dmas(nc)
    mybir.codegen_inst_isa_subclasses(nc)
    return nc


def kernel(Re_s, Im_s, omega, PiT, B, _trace=False):
    Re_s = np.ascontiguousarray(Re_s, dtype=np.float32)
    Im_s = np.ascontiguousarray(Im_s, dtype=np.float32)
    omega = np.ascontiguousarray(omega, dtype=np.float32)
    p = float(np.asarray(PiT).reshape(-1)[0])
    n = N_LAYERS
    tt_aa, qgw, ysq = _register_custom_ops()
    c1, kt, ktd, S0, e, d, inv1 = _host_scalars(np.asarray(B), p)

    nc = _build_bass(tt_aa, qgw, ysq, d, kt, ktd, S0)

    m64 = 2.0 * DEL_Z * omega.astype(np.float64)
    U0 = (Re_s.astype(np.float64) + inv1[0]).astype(np.float32)
    Y0 = (Im_s.astype(np.float64) * m64).astype(np.float32)
    Wf = (0.5 * m64 * m64).astype(np.float32)
    U08 = U0.reshape(N_CORES, P, F)
    Y08 = Y0.reshape(N_CORES, P, F)
    W8 = Wf.reshape(N_CORES, P, F)
    xin = np.concatenate([U08, Y08, W8], axis=2)  # [8, P, 3F]
    in_maps = [{"x_in": np.ascontiguousarray(xin[i])} for i in range(N_CORES)]
    res = run_bass_kernel_spmd(nc, in_maps, list(range(N_CORES)), trace=_trace)
    t1_full = np.concatenate(
        [res.results[i]["x_out"][:, 0:F].reshape(-1) for i in range(N_CORES)]
    )
    yc_full = np.concatenate(
        [res.results[i]["x_out"][:, F : 2 * F].reshape(-1) for i in range(N_CORES)]
    )
    re_full = t1_full.astype(np.float64) + ktd[n - 1]
    im_full = (yc_full.astype(np.float64) + e[n]) / m64
    if _trace:
        kernel.last_results = res
    return re_full.astype(np.float32), im_full.astype(np.float32)
